# revision 3
# baseline (speedup 1.0000x reference)
"""Trainium2 Bass kernel for nn_CharLevelModel (token->char scatter + MLP head).

Math: reference computes
  X  = concat(h0,h1) @ W_tok + b_tok          [B,T,2D]
  tok[b,c] = last token t whose char span [lo,hi) covers c (else -1)
  G  = X[b, tok] (0 where invalid)            [B,C,2D]
  out = (G @ W1 + b1) @ W2 + b2               [B,C,2]

Everything is linear, and row-gather commutes with the per-row linear maps, so
with A = W_tok @ W1 @ W2 ([2D,2]):
  out[b,c,:] = valid * (concat[b,tok[b,c],:] @ A + b_tok@W1@W2) + (b1@W2 + b2)

On device (per core, B/8 batches, all fp32):
  mask[t,c]   = (lo[t] <= c) & (c < hi[t])            [T=256 part-tiles, C=141]
  suffix[t,c] = sum_{t'>t} mask[t',c]                 (matmul w/ strict-triu ones)
  sel[t,c]    = mask * (suffix == 0)                  (<=1 one per column c)
  BgathT      = concat.T @ sel   via matmul(lhsT=concat_tile, rhs=sel)  [2D,C]
  outT[2,C]   = A.T @ BgathT     via matmul(lhsT=A_tile, rhs=BgathT_tile)
No transposes needed anywhere; both matmul stages consume natural layouts.
"""

import numpy as np

_B, _T, _D, _C = 64, 256, 768, 141
_NCORES = 8
_BPC = _B // _NCORES  # batches per core
_KD = (2 * _D) // 128  # 12 d-tiles of 128

_CACHE = {}


def _build(delta, bias_inv):
    """Build + compile the SPMD Bass program. delta/bias_inv are length-2
    float tuples baked into the NEFF (zero for this problem's setup)."""
    import concourse.bass as bass
    import concourse.tile as tile
    from concourse import bacc, mybir

    f32, i32 = mybir.dt.float32, mybir.dt.int32
    nc = bacc.Bacc("TRN2", target_bir_lowering=False, debug=False,
                   num_devices=_NCORES)
    h0_d = nc.dram_tensor("h0", [_BPC, _T, _D], f32, kind="ExternalInput")
    h1_d = nc.dram_tensor("h1", [_BPC, _T, _D], f32, kind="ExternalInput")
    offs_d = nc.dram_tensor("offs", [_BPC, _T, 2], i32, kind="ExternalInput")
    a_d = nc.dram_tensor("A", [2 * _D, 2], f32, kind="ExternalInput")
    out_d = nc.dram_tensor("out", [_BPC, 2, _C], f32, kind="ExternalOutput")

    with tile.TileContext(nc) as tc:
        with (
            tc.tile_pool(name="consts", bufs=1) as consts,
            tc.tile_pool(name="hid", bufs=8) as hid_pool,
            tc.tile_pool(name="offs", bufs=4) as offs_pool,
            tc.tile_pool(name="work", bufs=2) as work,
            tc.tile_pool(name="gath", bufs=24) as gath_pool,
            tc.tile_pool(name="outp", bufs=2) as out_pool,
            tc.tile_pool(name="ps_s", bufs=2, space="PSUM") as ps_s,
            tc.tile_pool(name="ps_g", bufs=3, space="PSUM") as ps_g,
            tc.tile_pool(name="ps_o", bufs=2, space="PSUM") as ps_o,
        ):
            # ---- constants (comparisons need f32 operands) ----
            c_iota_i = consts.tile([128, _C], i32)    # value = c (free idx)
            nc.gpsimd.iota(c_iota_i[:], pattern=[[1, _C]], channel_multiplier=0)
            c_iota = consts.tile([128, _C], f32)
            nc.gpsimd.tensor_copy(c_iota[:], c_iota_i[:])
            p_iota_i = consts.tile([128, 1], i32)     # value = partition idx
            nc.gpsimd.iota(p_iota_i[:], pattern=[[0, 1]], channel_multiplier=1)
            p_iota = consts.tile([128, 1], f32)
            nc.gpsimd.tensor_copy(p_iota[:], p_iota_i[:])
            f_iota_i = consts.tile([128, 128], i32)   # value = free idx
            nc.gpsimd.iota(f_iota_i[:], pattern=[[1, 128]], channel_multiplier=0)
            f_iota = consts.tile([128, 128], f32)
            nc.gpsimd.tensor_copy(f_iota[:], f_iota_i[:])
            # strict upper-tri ones: TRIU[t',t] = 1 iff t' > t  (t'=partition)
            triu = consts.tile([128, 128], f32)
            nc.gpsimd.tensor_scalar(triu[:], f_iota[:], p_iota[:], None,
                                    mybir.AluOpType.is_lt)
            ones = consts.tile([128, 128], f32)
            nc.vector.memset(ones[:], 1.0)
            a_sb = consts.tile([128, _KD, 2], f32)    # A[k*128+p, n] -> [p,k,n]
            nc.sync.dma_start(
                a_sb[:], a_d[:].rearrange("(k p) n -> p k n", p=128))
            use_delta = any(v != 0.0 for v in delta)
            if use_delta:
                dl = consts.tile([128, 2], f32)
                nc.vector.memset(dl[:, 0:1], float(delta[0]))
                nc.vector.memset(dl[:, 1:2], float(delta[1]))
            if any(v != 0.0 for v in bias_inv):
                bi = consts.tile([2, 1], f32)
                nc.vector.memset(bi[0:1, :], float(bias_inv[0]))
                nc.vector.memset(bi[1:2, :], float(bias_inv[1]))
            else:
                bi = None

            op = mybir.AluOpType
            for b in range(_BPC):
                # ---- load inputs ----
                offs_t = []
                for ti in range(2):
                    ot = offs_pool.tile([128, 2], i32, tag="offs")
                    nc.sync.dma_start(ot[:], offs_d[b, ti * 128:(ti + 1) * 128, :])
                    of = offs_pool.tile([128, 2], f32, tag="offsf")
                    nc.gpsimd.tensor_copy(of[:], ot[:])
                    offs_t.append(of)
                hid = []  # hid[ti] = [128, 1536] view pieces (h0 | h1)
                for ti in range(2):
                    t0 = hid_pool.tile([128, _D], f32, tag="hid")
                    nc.sync.dma_start(t0[:], h0_d[b, ti * 128:(ti + 1) * 128, :])
                    t1 = hid_pool.tile([128, _D], f32, tag="hid")
                    nc.sync.dma_start(t1[:], h1_d[b, ti * 128:(ti + 1) * 128, :])
                    hid.append((t0, t1))

                # ---- mask[t,c] ----
                masks = []
                for ti in range(2):
                    ge = work.tile([128, _C], f32, tag="ge")
                    nc.gpsimd.tensor_scalar(ge[:], c_iota[:],
                                            offs_t[ti][:, 0:1], None, op.is_ge)
                    lt = work.tile([128, _C], f32, tag="lt")
                    nc.gpsimd.tensor_scalar(lt[:], c_iota[:],
                                            offs_t[ti][:, 1:2], None, op.is_lt)
                    mk = work.tile([128, _C], f32, tag="mask")
                    nc.vector.tensor_tensor(mk[:], ge[:], lt[:], op.mult)
                    masks.append(mk)

                # ---- suffix counts via matmul, then sel ----
                s0 = ps_s.tile([128, _C], f32, tag="suf")
                nc.tensor.matmul(s0[:], triu[:], masks[0][:], start=True, stop=False)
                nc.tensor.matmul(s0[:], ones[:], masks[1][:], start=False, stop=True)
                s1 = ps_s.tile([128, _C], f32, tag="suf")
                nc.tensor.matmul(s1[:], triu[:], masks[1][:], start=True, stop=True)
                sels = []
                for ti, s in ((0, s0), (1, s1)):
                    eq = work.tile([128, _C], f32, tag="eq")
                    nc.vector.tensor_scalar(eq[:], s[:], 0.0, None, op.is_equal)
                    sl = work.tile([128, _C], f32, tag="sel")
                    nc.vector.tensor_tensor(sl[:], eq[:], masks[ti][:], op.mult)
                    sels.append(sl)

                # ---- gather: BgathT[d,c] = concat.T @ sel ----
                bg = []
                for mi in range(_KD):
                    half, ds = divmod(mi, 6)
                    g = ps_g.tile([128, _C], f32, tag="g")
                    for ti in range(2):
                        nc.tensor.matmul(
                            g[:], hid[ti][half][:, ds * 128:(ds + 1) * 128],
                            sels[ti][:], start=(ti == 0), stop=(ti == 1))
                    bgt = gath_pool.tile([128, _C], f32, tag="bg")
                    if mi % 2 == 0:
                        nc.vector.tensor_copy(bgt[:], g[:])
                    else:
                        nc.scalar.copy(bgt[:], g[:])
                    bg.append(bgt)

                # ---- project: outT[2,C] = A.T @ BgathT (+ delta * valid) ----
                o_ps = ps_o.tile([2, _C], f32, tag="o")
                n_mm = _KD + (2 if use_delta else 0)
                for mi in range(_KD):
                    nc.tensor.matmul(o_ps[:], a_sb[:, mi, :], bg[mi][:],
                                     start=(mi == 0), stop=(mi == n_mm - 1))
                if use_delta:
                    for ti in range(2):
                        nc.tensor.matmul(o_ps[:], dl[:], sels[ti][:],
                                         start=False, stop=(ti == 1))
                o_sb = out_pool.tile([2, _C], f32, tag="osb")
                if bi is None:
                    nc.vector.tensor_copy(o_sb[:], o_ps[:])
                else:
                    nc.vector.tensor_scalar(o_sb[:], o_ps[:], bi[:, 0:1], None,
                                            op.add)
                nc.sync.dma_start(out_d[b, :, :], o_sb[:])

    nc.compile()
    return nc


def kernel(hidden0, hidden1, offset_mapping, W_tok, b_tok, W1, b1, W2, b2,
           hidden_state):
    from concourse.bass_utils import run_bass_kernel_spmd

    # fold the three linear layers into one [2D, 2] map (exactly linear)
    w12 = W1.astype(np.float64) @ W2.astype(np.float64)        # [2D, 2]
    a_full = (W_tok.astype(np.float64) @ w12).astype(np.float32)
    delta = tuple(float(x) for x in (b_tok.astype(np.float64) @ w12))
    bias_inv = tuple(float(x) for x in
                     (b1.astype(np.float64) @ W2.astype(np.float64)
                      + b2.astype(np.float64)))

    key = (delta, bias_inv)
    if key not in _CACHE:
        _CACHE[key] = _build(delta, bias_inv)
    nc = _CACHE[key]

    in_maps = []
    for i in range(_NCORES):
        sl = slice(i * _BPC, (i + 1) * _BPC)
        in_maps.append({
            "h0": np.ascontiguousarray(hidden0[sl], dtype=np.float32),
            "h1": np.ascontiguousarray(hidden1[sl], dtype=np.float32),
            "offs": np.ascontiguousarray(offset_mapping[sl], dtype=np.int32),
            "A": a_full,
        })
    res = run_bass_kernel_spmd(nc, in_maps, core_ids=list(range(_NCORES)))

    out = np.empty((_B, 2, _C), np.float32)
    for i in range(_NCORES):
        out[i * _BPC:(i + 1) * _BPC] = res.results[i]["out"]
    start = np.ascontiguousarray(out[:, 0, :, None])
    end = np.ascontiguousarray(out[:, 1, :, None])
    return start, end, np.asarray(hidden_state)


# revision 8
# speedup vs baseline: 1.0462x; 1.0462x over previous
"""Trainium2 Bass kernel for nn_CharLevelModel (token->char scatter + MLP head).

Math: reference computes
  X  = concat(h0,h1) @ W_tok + b_tok          [B,T,2D]
  tok[b,c] = last token t whose char span [lo,hi) covers c (else -1)
  G  = X[b, tok] (0 where invalid)            [B,C,2D]
  out = (G @ W1 + b1) @ W2 + b2               [B,C,2]

Everything is linear, and row-gather commutes with the per-row linear maps, so
with A = W_tok @ W1 @ W2 ([2D,2]):
  out[b,c,:] = valid * (concat[b,tok[b,c],:] @ A + b_tok@W1@W2) + (b1@W2 + b2)

On device (per core, B/8 batches, all fp32):
  mask[t,c]   = (lo[t] <= c) & (c < hi[t])            [T=256 part-tiles, C=141]
  suffix[t,c] = sum_{t'>t} mask[t',c]                 (matmul w/ strict-triu ones)
  sel[t,c]    = mask * (suffix == 0)                  (<=1 one per column c)
  BgathT      = concat.T @ sel   via matmul(lhsT=concat_tile, rhs=sel)  [2D,C]
  outT[2,C]   = A.T @ BgathT     via matmul(lhsT=A_tile, rhs=BgathT_tile)
No transposes needed anywhere; both matmul stages consume natural layouts.
"""

import numpy as np

_B, _T, _D, _C = 64, 256, 768, 141
_NCORES = 8
_BPC = _B // _NCORES  # batches per core
_KD = (2 * _D) // 128  # 12 d-tiles of 128

_CACHE = {}


def _build(delta, bias_inv):
    """Build + compile the SPMD Bass program. delta/bias_inv are length-2
    float tuples baked into the NEFF (zero for this problem's setup)."""
    import concourse.bass as bass
    import concourse.tile as tile
    from concourse import bacc, mybir

    f32, i32 = mybir.dt.float32, mybir.dt.int32
    nc = bacc.Bacc("TRN2", target_bir_lowering=False, debug=False,
                   num_devices=_NCORES)
    h0_d = nc.dram_tensor("h0", [_BPC, _T, _D], f32, kind="ExternalInput")
    h1_d = nc.dram_tensor("h1", [_BPC, _T, _D], f32, kind="ExternalInput")
    offs_d = nc.dram_tensor("offs", [_BPC, _T, 2], i32, kind="ExternalInput")
    a_d = nc.dram_tensor("A", [2 * _D, 2], f32, kind="ExternalInput")
    out_d = nc.dram_tensor("out", [_BPC, 2, _C], f32, kind="ExternalOutput")

    with tile.TileContext(nc) as tc:
        with (
            tc.tile_pool(name="consts", bufs=1) as consts,
            tc.tile_pool(name="hid", bufs=6) as hid_pool,
            tc.tile_pool(name="offs", bufs=3) as offs_pool,
            tc.tile_pool(name="work", bufs=2) as work,
            tc.tile_pool(name="gath", bufs=24) as gath_pool,
            tc.tile_pool(name="outp", bufs=2) as out_pool,
            tc.tile_pool(name="ps_s", bufs=2, space="PSUM") as ps_s,
            tc.tile_pool(name="ps_g", bufs=3, space="PSUM") as ps_g,
            tc.tile_pool(name="ps_o", bufs=2, space="PSUM") as ps_o,
        ):
            # ---- constants (comparisons need f32 operands) ----
            c_iota_i = consts.tile([128, _C], i32)    # value = c (free idx)
            nc.gpsimd.iota(c_iota_i[:], pattern=[[1, _C]], channel_multiplier=0)
            c_iota = consts.tile([128, _C], f32)
            nc.gpsimd.tensor_copy(c_iota[:], c_iota_i[:])
            p_iota_i = consts.tile([128, 1], i32)     # value = partition idx
            nc.gpsimd.iota(p_iota_i[:], pattern=[[0, 1]], channel_multiplier=1)
            p_iota = consts.tile([128, 1], f32)
            nc.gpsimd.tensor_copy(p_iota[:], p_iota_i[:])
            f_iota_i = consts.tile([128, 128], i32)   # value = free idx
            nc.gpsimd.iota(f_iota_i[:], pattern=[[1, 128]], channel_multiplier=0)
            f_iota = consts.tile([128, 128], f32)
            nc.gpsimd.tensor_copy(f_iota[:], f_iota_i[:])
            # strict upper-tri ones: TRIU[t',t] = 1 iff t' > t  (t'=partition)
            triu = consts.tile([128, 128], f32)
            nc.gpsimd.tensor_scalar(triu[:], f_iota[:], p_iota[:], None,
                                    mybir.AluOpType.is_lt)
            ones = consts.tile([128, 128], f32)
            nc.vector.memset(ones[:], 1.0)
            a_sb = consts.tile([128, _KD, 2], f32)    # A[k*128+p, n] -> [p,k,n]
            nc.sync.dma_start(
                a_sb[:], a_d[:].rearrange("(k p) n -> p k n", p=128))
            use_delta = any(v != 0.0 for v in delta)
            if use_delta:
                dl = consts.tile([128, 2], f32)
                nc.vector.memset(dl[:, 0:1], float(delta[0]))
                nc.vector.memset(dl[:, 1:2], float(delta[1]))
            if any(v != 0.0 for v in bias_inv):
                bi = consts.tile([2, 1], f32)
                nc.vector.memset(bi[0:1, :], float(bias_inv[0]))
                nc.vector.memset(bi[1:2, :], float(bias_inv[1]))
            else:
                bi = None

            op = mybir.AluOpType
            for b in range(_BPC):
                # ---- load inputs (one big DMA per tensor per batch;
                #      spread issue across engines' queues) ----
                ot = offs_pool.tile([128, 2, 2], i32, tag="offs")
                nc.gpsimd.dma_start(
                    ot[:], offs_d[b].rearrange("(ti p) k -> p ti k", p=128))
                of = offs_pool.tile([128, 2, 2], f32, tag="offsf")
                nc.vector.tensor_copy(of[:], ot[:])
                h0_t = hid_pool.tile([128, 2, _D], f32, tag="hid")
                nc.sync.dma_start(
                    h0_t[:], h0_d[b].rearrange("(ti p) d -> p ti d", p=128))
                h1_t = hid_pool.tile([128, 2, _D], f32, tag="hid")
                nc.scalar.dma_start(
                    h1_t[:], h1_d[b].rearrange("(ti p) d -> p ti d", p=128))

                # ---- mask[t,c] ----
                masks = []
                for ti in range(2):
                    ge = work.tile([128, _C], f32, tag="ge")
                    nc.vector.tensor_scalar(ge[:], c_iota[:],
                                            of[:, ti, 0:1], None, op.is_ge)
                    lt = work.tile([128, _C], f32, tag="lt")
                    nc.vector.tensor_scalar(lt[:], c_iota[:],
                                            of[:, ti, 1:2], None, op.is_lt)
                    mk = work.tile([128, _C], f32, tag="mask")
                    nc.vector.tensor_tensor(mk[:], ge[:], lt[:], op.mult)
                    masks.append(mk)

                # ---- suffix counts via matmul, then sel ----
                s0 = ps_s.tile([128, _C], f32, tag="suf")
                nc.tensor.matmul(s0[:], triu[:], masks[0][:], start=True, stop=False)
                nc.tensor.matmul(s0[:], ones[:], masks[1][:], start=False, stop=True)
                s1 = ps_s.tile([128, _C], f32, tag="suf")
                nc.tensor.matmul(s1[:], triu[:], masks[1][:], start=True, stop=True)
                sels = []
                for ti, s in ((0, s0), (1, s1)):
                    eq = work.tile([128, _C], f32, tag="eq")
                    nc.vector.tensor_scalar(eq[:], s[:], 0.0, None, op.is_equal)
                    sl = work.tile([128, _C], f32, tag="sel")
                    nc.vector.tensor_tensor(sl[:], eq[:], masks[ti][:], op.mult)
                    sels.append(sl)

                # ---- gather: BgathT[d,c] = concat.T @ sel ----
                bg = []
                for mi in range(_KD):
                    half, ds = divmod(mi, 6)
                    ht = h0_t if half == 0 else h1_t
                    g = ps_g.tile([128, _C], f32, tag="g")
                    for ti in range(2):
                        nc.tensor.matmul(
                            g[:], ht[:, ti, ds * 128:(ds + 1) * 128],
                            sels[ti][:], start=(ti == 0), stop=(ti == 1))
                    bgt = gath_pool.tile([128, _C], f32, tag="bg")
                    if mi % 2 == 0:
                        nc.vector.tensor_copy(bgt[:], g[:])
                    else:
                        nc.scalar.copy(bgt[:], g[:])
                    bg.append(bgt)

                # ---- project: outT[2,C] = A.T @ BgathT (+ delta * valid) ----
                o_ps = ps_o.tile([2, _C], f32, tag="o")
                n_mm = _KD + (2 if use_delta else 0)
                for mi in range(_KD):
                    nc.tensor.matmul(o_ps[:], a_sb[:, mi, :], bg[mi][:],
                                     start=(mi == 0), stop=(mi == n_mm - 1))
                if use_delta:
                    for ti in range(2):
                        nc.tensor.matmul(o_ps[:], dl[:], sels[ti][:],
                                         start=False, stop=(ti == 1))
                o_sb = out_pool.tile([2, _C], f32, tag="osb")
                if bi is None:
                    nc.vector.tensor_copy(o_sb[:], o_ps[:])
                else:
                    nc.vector.tensor_scalar(o_sb[:], o_ps[:], bi[:, 0:1], None,
                                            op.add)
                nc.gpsimd.dma_start(out_d[b, :, :], o_sb[:])

    nc.compile()
    return nc


def kernel(hidden0, hidden1, offset_mapping, W_tok, b_tok, W1, b1, W2, b2,
           hidden_state):
    from concourse.bass_utils import run_bass_kernel_spmd

    # fold the three linear layers into one [2D, 2] map (exactly linear)
    w12 = W1.astype(np.float64) @ W2.astype(np.float64)        # [2D, 2]
    a_full = (W_tok.astype(np.float64) @ w12).astype(np.float32)
    delta = tuple(float(x) for x in (b_tok.astype(np.float64) @ w12))
    bias_inv = tuple(float(x) for x in
                     (b1.astype(np.float64) @ W2.astype(np.float64)
                      + b2.astype(np.float64)))

    key = (delta, bias_inv)
    if key not in _CACHE:
        _CACHE[key] = _build(delta, bias_inv)
    nc = _CACHE[key]

    in_maps = []
    for i in range(_NCORES):
        sl = slice(i * _BPC, (i + 1) * _BPC)
        in_maps.append({
            "h0": np.ascontiguousarray(hidden0[sl], dtype=np.float32),
            "h1": np.ascontiguousarray(hidden1[sl], dtype=np.float32),
            "offs": np.ascontiguousarray(offset_mapping[sl], dtype=np.int32),
            "A": a_full,
        })
    res = run_bass_kernel_spmd(nc, in_maps, core_ids=list(range(_NCORES)))

    out = np.empty((_B, 2, _C), np.float32)
    for i in range(_NCORES):
        out[i * _BPC:(i + 1) * _BPC] = res.results[i]["out"]
    start = np.ascontiguousarray(out[:, 0, :, None])
    end = np.ascontiguousarray(out[:, 1, :, None])
    return start, end, np.asarray(hidden_state)


# revision 9
# speedup vs baseline: 1.6835x; 1.6093x over previous
"""Trainium2 Bass kernel for nn_CharLevelModel (token->char scatter + MLP head).

Math: reference computes
  X  = concat(h0,h1) @ W_tok + b_tok          [B,T,2D]
  tok[b,c] = last token t whose char span [lo,hi) covers c (else -1)
  G  = X[b, tok] (0 where invalid)            [B,C,2D]
  out = (G @ W1 + b1) @ W2 + b2               [B,C,2]

Everything is linear, and row-gather commutes with the per-row linear maps, so
with A = W_tok @ W1 @ W2 ([2D,2]):
  out[b,c,:] = valid * (concat[b,tok[b,c],:] @ A + b_tok@W1@W2) + (b1@W2 + b2)

On device (per core, B/8 batches; matmuls in bf16, accumulate fp32):
  mask[t,c]   = (lo[t] <= c) & (c < hi[t])            [T=256 part-tiles, C=141]
  suffix[t,c] = sum_{t'>t} mask[t',c]                 (matmul w/ strict-triu ones)
  sel[t,c]    = mask * (suffix == 0)                  (<=1 one per column c)
  BgathT      = concat.T @ sel   via matmul(lhsT=concat_tile, rhs=sel)  [2D,C]
  outT[2,C]   = A.T @ BgathT     via matmul(lhsT=A_tile, rhs=BgathT_tile)
No transposes needed anywhere; both matmul stages consume natural layouts.
sel is exactly 0/1 (bf16-exact), so the gather matmul reproduces the bf16
hidden values exactly; only the bf16 input rounding and the A-projection
rounding affect accuracy (~1e-3 rel).
"""

import numpy as np

_B, _T, _D, _C = 64, 256, 768, 141
_NCORES = 8
_BPC = _B // _NCORES  # batches per core
_KD = (2 * _D) // 128  # 12 d-tiles of 128

_CACHE = {}


def _build(delta, bias_inv):
    """Build + compile the SPMD Bass program. delta/bias_inv are length-2
    float tuples baked into the NEFF (zero for this problem's setup)."""
    import concourse.bass as bass
    import concourse.tile as tile
    from concourse import bacc, mybir

    f32, i32 = mybir.dt.float32, mybir.dt.int32
    bf16 = mybir.dt.bfloat16
    nc = bacc.Bacc("TRN2", target_bir_lowering=False, debug=False,
                   num_devices=_NCORES)
    h0_d = nc.dram_tensor("h0", [_BPC, _T, _D], bf16, kind="ExternalInput")
    h1_d = nc.dram_tensor("h1", [_BPC, _T, _D], bf16, kind="ExternalInput")
    offs_d = nc.dram_tensor("offs", [_BPC, _T, 2], i32, kind="ExternalInput")
    a_d = nc.dram_tensor("A", [2 * _D, 2], bf16, kind="ExternalInput")
    out_d = nc.dram_tensor("out", [_BPC, 2, _C], f32, kind="ExternalOutput")

    with tile.TileContext(nc) as tc:
        with (
            tc.tile_pool(name="consts", bufs=1) as consts,
            tc.tile_pool(name="hid", bufs=6) as hid_pool,
            tc.tile_pool(name="offs", bufs=3) as offs_pool,
            tc.tile_pool(name="work", bufs=3) as work,
            tc.tile_pool(name="gath", bufs=24) as gath_pool,
            tc.tile_pool(name="outp", bufs=2) as out_pool,
            tc.tile_pool(name="ps_s", bufs=2, space="PSUM") as ps_s,
            tc.tile_pool(name="ps_g", bufs=3, space="PSUM") as ps_g,
            tc.tile_pool(name="ps_o", bufs=2, space="PSUM") as ps_o,
        ):
            # ---- constants (tensor_scalar comparisons need f32 scalars) ----
            c_iota_i = consts.tile([128, _C], i32)    # value = c (free idx)
            nc.gpsimd.iota(c_iota_i[:], pattern=[[1, _C]], channel_multiplier=0)
            c_iota = consts.tile([128, _C], f32)
            nc.gpsimd.tensor_copy(c_iota[:], c_iota_i[:])
            p_iota_i = consts.tile([128, 1], i32)     # value = partition idx
            nc.gpsimd.iota(p_iota_i[:], pattern=[[0, 1]], channel_multiplier=1)
            p_iota = consts.tile([128, 1], f32)
            nc.gpsimd.tensor_copy(p_iota[:], p_iota_i[:])
            f_iota_i = consts.tile([128, 128], i32)   # value = free idx
            nc.gpsimd.iota(f_iota_i[:], pattern=[[1, 128]], channel_multiplier=0)
            f_iota = consts.tile([128, 128], f32)
            nc.gpsimd.tensor_copy(f_iota[:], f_iota_i[:])
            # strict upper-tri ones: TRIU[t',t] = 1 iff t' > t  (t'=partition)
            triu = consts.tile([128, 128], bf16)
            nc.gpsimd.tensor_scalar(triu[:], f_iota[:], p_iota[:], None,
                                    mybir.AluOpType.is_lt)
            ones = consts.tile([128, 128], bf16)
            nc.vector.memset(ones[:], 1.0)
            a_sb = consts.tile([128, _KD, 2], bf16)   # A[k*128+p, n] -> [p,k,n]
            nc.sync.dma_start(
                a_sb[:], a_d[:].rearrange("(k p) n -> p k n", p=128))
            use_delta = any(v != 0.0 for v in delta)
            if use_delta:
                dl = consts.tile([128, 2], bf16)
                nc.vector.memset(dl[:, 0:1], float(delta[0]))
                nc.vector.memset(dl[:, 1:2], float(delta[1]))
            if any(v != 0.0 for v in bias_inv):
                bi = consts.tile([2, 1], f32)
                nc.vector.memset(bi[0:1, :], float(bias_inv[0]))
                nc.vector.memset(bi[1:2, :], float(bias_inv[1]))
            else:
                bi = None

            op = mybir.AluOpType
            for b in range(_BPC):
                # ---- load inputs (one big DMA per tensor per batch;
                #      spread issue across engines' queues) ----
                ot = offs_pool.tile([128, 2, 2], i32, tag="offs")
                nc.gpsimd.dma_start(
                    ot[:], offs_d[b].rearrange("(ti p) k -> p ti k", p=128))
                of = offs_pool.tile([128, 2, 2], f32, tag="offsf")
                nc.vector.tensor_copy(of[:], ot[:])
                h0_t = hid_pool.tile([128, 2, _D], bf16, tag="hid")
                nc.sync.dma_start(
                    h0_t[:], h0_d[b].rearrange("(ti p) d -> p ti d", p=128))
                h1_t = hid_pool.tile([128, 2, _D], bf16, tag="hid")
                nc.scalar.dma_start(
                    h1_t[:], h1_d[b].rearrange("(ti p) d -> p ti d", p=128))

                # ---- mask[t,c] (bf16 0/1) ----
                masks = []
                for ti in range(2):
                    ge = work.tile([128, _C], bf16, tag="ge")
                    nc.vector.tensor_scalar(ge[:], c_iota[:],
                                            of[:, ti, 0:1], None, op.is_ge)
                    lt = work.tile([128, _C], bf16, tag="lt")
                    nc.vector.tensor_scalar(lt[:], c_iota[:],
                                            of[:, ti, 1:2], None, op.is_lt)
                    mk = work.tile([128, _C], bf16, tag="mask")
                    nc.vector.tensor_tensor(mk[:], ge[:], lt[:], op.mult)
                    masks.append(mk)

                # ---- suffix counts via matmul, then sel ----
                s0 = ps_s.tile([128, _C], f32, tag="suf")
                nc.tensor.matmul(s0[:], triu[:], masks[0][:], start=True, stop=False)
                nc.tensor.matmul(s0[:], ones[:], masks[1][:], start=False, stop=True)
                s1 = ps_s.tile([128, _C], f32, tag="suf")
                nc.tensor.matmul(s1[:], triu[:], masks[1][:], start=True, stop=True)
                sels = []
                for ti, s in ((0, s0), (1, s1)):
                    eq = work.tile([128, _C], bf16, tag="eq")
                    nc.vector.tensor_scalar(eq[:], s[:], 0.0, None, op.is_equal)
                    sl = work.tile([128, _C], bf16, tag="sel")
                    nc.vector.tensor_tensor(sl[:], eq[:], masks[ti][:], op.mult)
                    sels.append(sl)

                # ---- gather: BgathT[d,c] = concat.T @ sel ----
                bg = []
                for mi in range(_KD):
                    half, ds = divmod(mi, 6)
                    ht = h0_t if half == 0 else h1_t
                    g = ps_g.tile([128, _C], f32, tag="g")
                    for ti in range(2):
                        nc.tensor.matmul(
                            g[:], ht[:, ti, ds * 128:(ds + 1) * 128],
                            sels[ti][:], start=(ti == 0), stop=(ti == 1))
                    bgt = gath_pool.tile([128, _C], bf16, tag="bg")
                    if mi % 2 == 0:
                        nc.vector.tensor_copy(bgt[:], g[:])
                    else:
                        nc.scalar.copy(bgt[:], g[:])
                    bg.append(bgt)

                # ---- project: outT[2,C] = A.T @ BgathT (+ delta * valid) ----
                o_ps = ps_o.tile([2, _C], f32, tag="o")
                n_mm = _KD + (2 if use_delta else 0)
                for mi in range(_KD):
                    nc.tensor.matmul(o_ps[:], a_sb[:, mi, :], bg[mi][:],
                                     start=(mi == 0), stop=(mi == n_mm - 1))
                if use_delta:
                    for ti in range(2):
                        nc.tensor.matmul(o_ps[:], dl[:], sels[ti][:],
                                         start=False, stop=(ti == 1))
                o_sb = out_pool.tile([2, _C], f32, tag="osb")
                if bi is None:
                    nc.vector.tensor_copy(o_sb[:], o_ps[:])
                else:
                    nc.vector.tensor_scalar(o_sb[:], o_ps[:], bi[:, 0:1], None,
                                            op.add)
                nc.gpsimd.dma_start(out_d[b, :, :], o_sb[:])

    nc.compile()
    return nc


def kernel(hidden0, hidden1, offset_mapping, W_tok, b_tok, W1, b1, W2, b2,
           hidden_state):
    import ml_dtypes
    from concourse.bass_utils import run_bass_kernel_spmd

    # fold the three linear layers into one [2D, 2] map (exactly linear)
    w12 = W1.astype(np.float64) @ W2.astype(np.float64)        # [2D, 2]
    a_full = (W_tok.astype(np.float64) @ w12).astype(ml_dtypes.bfloat16)
    delta = tuple(float(x) for x in (b_tok.astype(np.float64) @ w12))
    bias_inv = tuple(float(x) for x in
                     (b1.astype(np.float64) @ W2.astype(np.float64)
                      + b2.astype(np.float64)))

    key = (delta, bias_inv)
    if key not in _CACHE:
        _CACHE[key] = _build(delta, bias_inv)
    nc = _CACHE[key]

    h0b = np.asarray(hidden0, np.float32).astype(ml_dtypes.bfloat16)
    h1b = np.asarray(hidden1, np.float32).astype(ml_dtypes.bfloat16)
    offs = np.ascontiguousarray(offset_mapping, dtype=np.int32)
    in_maps = []
    for i in range(_NCORES):
        sl = slice(i * _BPC, (i + 1) * _BPC)
        in_maps.append({
            "h0": np.ascontiguousarray(h0b[sl]),
            "h1": np.ascontiguousarray(h1b[sl]),
            "offs": offs[sl],
            "A": a_full,
        })
    res = run_bass_kernel_spmd(nc, in_maps, core_ids=list(range(_NCORES)))

    out = np.empty((_B, 2, _C), np.float32)
    for i in range(_NCORES):
        out[i * _BPC:(i + 1) * _BPC] = res.results[i]["out"]
    start = np.ascontiguousarray(out[:, 0, :, None])
    end = np.ascontiguousarray(out[:, 1, :, None])
    return start, end, np.asarray(hidden_state)


# revision 10
# speedup vs baseline: 2.1990x; 1.3062x over previous
"""Trainium2 Bass kernel for nn_CharLevelModel (token->char scatter + MLP head).

Math: reference computes
  X  = concat(h0,h1) @ W_tok + b_tok          [B,T,2D]
  tok[b,c] = last token t whose char span [lo,hi) covers c (else -1)
  G  = X[b, tok] (0 where invalid)            [B,C,2D]
  out = (G @ W1 + b1) @ W2 + b2               [B,C,2]

Everything is linear, and row-gather commutes with the per-row linear maps, so
with A = W_tok @ W1 @ W2 ([2D,2]):
  out[b,c,:] = valid * (concat[b,tok[b,c],:] @ A + b_tok@W1@W2) + (b1@W2 + b2)

On device (per core, B/8 batches; matmuls in bf16, accumulate fp32):
  mask[t,c]   = (lo[t] <= c) & (c < hi[t])            [T=256 tokens, C=141]
  suffix[t,c] = sum_{t'>t} mask[t',c]                 (matmul w/ triangular ones)
  sel[t,c]    = mask * (suffix == 0)                  (<=1 one per column c)
  BgathT      = concat.T @ sel   via matmul(lhsT=concat_tile, rhs=sel)  [2D,C]
  outT[2,C]   = A.T @ BgathT     via matmul(lhsT=A_tile, rhs=BgathT_tile)
No transposes needed anywhere; both matmul stages consume natural layouts.
sel is exactly 0/1 (bf16-exact) so the gather matmul reproduces the bf16
hidden values exactly; only the bf16 input/A rounding matters (~1e-3 rel).

Tokens are laid out interleaved across the two 128-partition tiles:
t = 2p + ti, so each SBUF partition p holds tokens (2p, 2p+1) and the DMA from
hidden[b] ([256, 768] row-major) is a single fully-contiguous transfer.
With this order the strict "t' > t" block matrices are:
  (ti'=0,ti=0) strict   (ti'=1,ti=0) inclusive
  (ti'=0,ti=1) strict   (ti'=1,ti=1) strict
so  S0 = strict@mask0 + incl@mask1,  S1 = strict@(mask0+mask1).
"""

import numpy as np

_B, _T, _D, _C = 64, 256, 768, 141
_NCORES = 8
_BPC = _B // _NCORES  # batches per core
_KD = (2 * _D) // 128  # 12 d-tiles of 128

_CACHE = {}
_CONSTS = {}


def _host_consts():
    if _CONSTS:
        return _CONSTS
    import ml_dtypes
    cio = np.broadcast_to(np.arange(_C, dtype=np.float32), (128, _C))
    p = np.arange(128)
    tr = np.empty((128, 2, 128), dtype=ml_dtypes.bfloat16)
    tr[:, 0, :] = (p[:, None] > p[None, :]).astype(ml_dtypes.bfloat16)
    tr[:, 1, :] = (p[:, None] >= p[None, :]).astype(ml_dtypes.bfloat16)
    _CONSTS["CIO"] = np.ascontiguousarray(cio)
    _CONSTS["TR"] = tr
    return _CONSTS


def _build(delta, bias_inv):
    """Build + compile the SPMD Bass program. delta/bias_inv are length-2
    float tuples baked into the NEFF (zero for this problem's setup)."""
    import concourse.bass as bass
    import concourse.tile as tile
    from concourse import bacc, mybir

    f32, i32 = mybir.dt.float32, mybir.dt.int32
    bf16 = mybir.dt.bfloat16
    nc = bacc.Bacc("TRN2", target_bir_lowering=False, debug=False,
                   num_devices=_NCORES)
    h0_d = nc.dram_tensor("h0", [_BPC, _T, _D], bf16, kind="ExternalInput")
    h1_d = nc.dram_tensor("h1", [_BPC, _T, _D], bf16, kind="ExternalInput")
    offs_d = nc.dram_tensor("offs", [_BPC, _T, 2], i32, kind="ExternalInput")
    a_d = nc.dram_tensor("A", [2 * _D, 2], bf16, kind="ExternalInput")
    cio_d = nc.dram_tensor("CIO", [128, _C], f32, kind="ExternalInput")
    tr_d = nc.dram_tensor("TR", [128, 2, 128], bf16, kind="ExternalInput")
    out_d = nc.dram_tensor("out", [_BPC, 2, _C], f32, kind="ExternalOutput")

    with tile.TileContext(nc) as tc:
        with (
            tc.tile_pool(name="consts", bufs=1) as consts,
            tc.tile_pool(name="hid", bufs=10) as hid_pool,
            tc.tile_pool(name="offs", bufs=3) as offs_pool,
            tc.tile_pool(name="work", bufs=3) as work,
            tc.tile_pool(name="gath", bufs=8) as gath_pool,
            tc.tile_pool(name="outp", bufs=2) as out_pool,
            tc.tile_pool(name="ps_s", bufs=2, space="PSUM") as ps_s,
            tc.tile_pool(name="ps_g", bufs=3, space="PSUM") as ps_g,
            tc.tile_pool(name="ps_o", bufs=2, space="PSUM") as ps_o,
        ):
            # ---- constants (host-built, DMA'd once) ----
            c_iota = consts.tile([128, _C], f32)
            nc.sync.dma_start(c_iota[:], cio_d[:])
            tr = consts.tile([128, 2, 128], bf16)  # [:,0]=strict, [:,1]=incl
            nc.sync.dma_start(tr[:], tr_d[:])
            a_sb = consts.tile([128, _KD, 2], bf16)  # A[k*128+p, n] -> [p,k,n]
            nc.sync.dma_start(
                a_sb[:], a_d[:].rearrange("(k p) n -> p k n", p=128))
            use_delta = any(v != 0.0 for v in delta)
            if use_delta:
                dl = consts.tile([128, 2], bf16)
                nc.vector.memset(dl[:, 0:1], float(delta[0]))
                nc.vector.memset(dl[:, 1:2], float(delta[1]))
            if any(v != 0.0 for v in bias_inv):
                bi = consts.tile([2, 1], f32)
                nc.vector.memset(bi[0:1, :], float(bias_inv[0]))
                nc.vector.memset(bi[1:2, :], float(bias_inv[1]))
            else:
                bi = None

            op = mybir.AluOpType
            for b in range(_BPC):
                # ---- load inputs; t = 2p + ti interleave keeps every DMA
                #      source fully contiguous ----
                ot = offs_pool.tile([128, 2, 2], i32, tag="offs")
                nc.gpsimd.dma_start(
                    ot[:], offs_d[b].rearrange("(p ti) k -> p ti k", ti=2))
                of = offs_pool.tile([128, 2, 2], f32, tag="offsf")
                nc.gpsimd.tensor_copy(of[:], ot[:])
                h0_t = hid_pool.tile([128, 2, _D], bf16, tag="hid")
                nc.sync.dma_start(
                    h0_t[:], h0_d[b].rearrange("(p ti) d -> p ti d", ti=2))
                h1_t = hid_pool.tile([128, 2, _D], bf16, tag="hid")
                nc.scalar.dma_start(
                    h1_t[:], h1_d[b].rearrange("(p ti) d -> p ti d", ti=2))

                # ---- mask[t,c] (bf16 0/1) ----
                masks = []
                for ti in range(2):
                    ge = work.tile([128, _C], bf16, tag="ge")
                    nc.vector.tensor_scalar(ge[:], c_iota[:],
                                            of[:, ti, 0:1], None, op.is_ge)
                    lt = work.tile([128, _C], bf16, tag="lt")
                    nc.vector.tensor_scalar(lt[:], c_iota[:],
                                            of[:, ti, 1:2], None, op.is_lt)
                    mk = work.tile([128, _C], bf16, tag="mask")
                    nc.vector.tensor_tensor(mk[:], ge[:], lt[:], op.mult)
                    masks.append(mk)
                msum = work.tile([128, _C], bf16, tag="msum")
                nc.vector.tensor_tensor(msum[:], masks[0][:], masks[1][:],
                                        op.add)

                # ---- suffix counts via matmul, then sel ----
                s0 = ps_s.tile([128, _C], f32, tag="suf")
                nc.tensor.matmul(s0[:], tr[:, 0, :], masks[0][:],
                                 start=True, stop=False)
                nc.tensor.matmul(s0[:], tr[:, 1, :], masks[1][:],
                                 start=False, stop=True)
                s1 = ps_s.tile([128, _C], f32, tag="suf")
                nc.tensor.matmul(s1[:], tr[:, 0, :], msum[:],
                                 start=True, stop=True)
                sels = []
                for ti, s in ((0, s0), (1, s1)):
                    eq = work.tile([128, _C], bf16, tag="eq")
                    nc.vector.tensor_scalar(eq[:], s[:], 0.0, None, op.is_equal)
                    sl = work.tile([128, _C], bf16, tag="sel")
                    nc.vector.tensor_tensor(sl[:], eq[:], masks[ti][:], op.mult)
                    sels.append(sl)

                # ---- gather: BgathT[d,c] = concat.T @ sel; 3 d-tiles per
                #      PSUM bank so PSUM->SBUF moves in 4 copies, not 12 ----
                bg = []
                for gi in range(4):
                    g = ps_g.tile([128, 3, _C], f32, tag="g")
                    for jj in range(3):
                        mi = gi * 3 + jj
                        half, ds = divmod(mi, 6)
                        ht = h0_t if half == 0 else h1_t
                        for ti in range(2):
                            nc.tensor.matmul(
                                g[:, jj, :], ht[:, ti, ds * 128:(ds + 1) * 128],
                                sels[ti][:], start=(ti == 0), stop=(ti == 1))
                    bgt = gath_pool.tile([128, 3, _C], bf16, tag="bg")
                    if gi % 2 == 0:
                        nc.vector.tensor_copy(bgt[:], g[:])
                    else:
                        nc.scalar.copy(bgt[:], g[:])
                    bg.append(bgt)

                # ---- project: outT[2,C] = A.T @ BgathT (+ delta * valid) ----
                o_ps = ps_o.tile([2, _C], f32, tag="o")
                n_mm = _KD + (2 if use_delta else 0)
                for mi in range(_KD):
                    nc.tensor.matmul(o_ps[:], a_sb[:, mi, :],
                                     bg[mi // 3][:, mi % 3, :],
                                     start=(mi == 0), stop=(mi == n_mm - 1))
                if use_delta:
                    for ti in range(2):
                        nc.tensor.matmul(o_ps[:], dl[:], sels[ti][:],
                                         start=False, stop=(ti == 1))
                o_sb = out_pool.tile([2, _C], f32, tag="osb")
                if bi is None:
                    nc.vector.tensor_copy(o_sb[:], o_ps[:])
                else:
                    nc.vector.tensor_scalar(o_sb[:], o_ps[:], bi[:, 0:1], None,
                                            op.add)
                nc.gpsimd.dma_start(out_d[b, :, :], o_sb[:])

    nc.compile()
    return nc


def _make_in_maps(hidden0, hidden1, offset_mapping, a_full):
    import ml_dtypes
    consts = _host_consts()
    h0b = np.asarray(hidden0, np.float32).astype(ml_dtypes.bfloat16)
    h1b = np.asarray(hidden1, np.float32).astype(ml_dtypes.bfloat16)
    offs = np.ascontiguousarray(offset_mapping, dtype=np.int32)
    in_maps = []
    for i in range(_NCORES):
        sl = slice(i * _BPC, (i + 1) * _BPC)
        in_maps.append({
            "h0": np.ascontiguousarray(h0b[sl]),
            "h1": np.ascontiguousarray(h1b[sl]),
            "offs": offs[sl],
            "A": a_full,
            "CIO": consts["CIO"],
            "TR": consts["TR"],
        })
    return in_maps


def _fold_weights(W_tok, b_tok, W1, b1, W2, b2):
    import ml_dtypes
    w12 = W1.astype(np.float64) @ W2.astype(np.float64)        # [2D, 2]
    a_full = (W_tok.astype(np.float64) @ w12).astype(ml_dtypes.bfloat16)
    delta = tuple(float(x) for x in (b_tok.astype(np.float64) @ w12))
    bias_inv = tuple(float(x) for x in
                     (b1.astype(np.float64) @ W2.astype(np.float64)
                      + b2.astype(np.float64)))
    return a_full, delta, bias_inv


def kernel(hidden0, hidden1, offset_mapping, W_tok, b_tok, W1, b1, W2, b2,
           hidden_state):
    from concourse.bass_utils import run_bass_kernel_spmd

    a_full, delta, bias_inv = _fold_weights(W_tok, b_tok, W1, b1, W2, b2)
    key = (delta, bias_inv)
    if key not in _CACHE:
        _CACHE[key] = _build(delta, bias_inv)
    nc = _CACHE[key]

    in_maps = _make_in_maps(hidden0, hidden1, offset_mapping, a_full)
    res = run_bass_kernel_spmd(nc, in_maps, core_ids=list(range(_NCORES)))

    out = np.empty((_B, 2, _C), np.float32)
    for i in range(_NCORES):
        out[i * _BPC:(i + 1) * _BPC] = res.results[i]["out"]
    start = np.ascontiguousarray(out[:, 0, :, None])
    end = np.ascontiguousarray(out[:, 1, :, None])
    return start, end, np.asarray(hidden_state)
